# revision 47
# baseline (speedup 1.0000x reference)
"""Trainium2 Bass kernel for nn_Attention_15676630631260 (sparse_attention).

reference:
  q = x @ Wq.T + bq ; k = x @ Wk.T + bk ; v = x @ Wv.T + bv        (per batch)
  scores = sigmoid(q @ k.T / sqrt(P))                               [B,S,S]
  out[b,i,j,:] = tril(i,j) * scores[b,i,j] * v[b,j,:]               [B,S,S,P]

B=2, S=512, D=256, P=128.  Output is 256 MB; the causal mask zeroes the
j>i region.  run_bass_kernel_spmd pre-zeroes ExternalOutput buffers
(donated zero buffers under PJRT), so the kernel only writes the j<=i
region — at 128-column tile granularity per row: row i writes j-tiles
0..i//128 (the partial diagonal tile is zeroed exactly via a
host-supplied mask).

Sharding (8 cores, one NEFF, SPMD): core c -> batch b=c//4, quarter
k=c%4.  Rows are assigned as 8-row groups: local group g=4t+s holds
global rows 128t+32s+8k..+8, so local group slot s always needs only
j < 32(s+1) of its diagonal tile on EVERY core -> the diagonal tile is
written trimmed to that fixed partition allowance (identical
instruction stream, 17.0 MiB written per core instead of 20 MiB; the
j>=allowance region stays pre-zeroed).

Per-core device program (all matmuls fp32r: single-pass, ~tf32-grade):
  Q^T[p,i]; K^T/V^T/scores per j-tile.  Tiles 0 and 1 take a "narrow"
  fast path whose inputs come from ONE packed critical-input DMA (one
  first-byte latency); tiles 2-3 take a wide N=256 path.  scores are
  computed as [i,j], sigmoid'd on ACT, PE-transposed to [j,i], masked
  on DVE.  V^T gets its bias on the PSUM->SBUF copy (per-partition
  tensor_scalar add) and is PE-transposed to V[s,p].
  Output rows are produced as [j_partition, (jt, i, p)] slabs:
  broadcast row-scaling of V by score columns, batched 8 rows per DVE
  tensor_tensor (stride-0 broadcast APs), with a spread subset done as
  per-row activation-scale ops on ACT to balance the engines; then
  batched HWDGE DMAs ([j, jt, (i p)] — 4 KB contiguous runs per
  partition) into the [j, i_local, p]-layout local output shard.
Classes stream in order 0, 1, 3, 2 (fast ramp, small tail).
"""

import os
import sys

import numpy as np

for _p in ("/root/.axon_site/_ro/trn_rl_repo", "/opt/trn_rl_repo"):
    if _p not in sys.path and os.path.isdir(_p):
        sys.path.append(_p)

import concourse.bass as bass
import concourse.mybir as mybir
from concourse.tile import TileContext
from concourse.masks import make_identity
from concourse import bass_utils

F32 = mybir.dt.float32
F32R = mybir.dt.float32r
B, S, D, P = 2, 512, 256, 128
NCORES = 8
GROUP = 8           # output rows per DMA group
NGROUPS = 128 // GROUP
INV_SQRT_P = float(1.0 / np.sqrt(np.float32(P)))
# packed critical-input pieces (four DMAs on the sync ring, in order),
# staged so each dependency chain unblocks at the earliest possible sem:
#   a1: wk(2x128) xt0(2x128) b3(3)   -> K-path (first matmuls)
#   a2: wq(2x128) xq(2x128)          -> Q path
#   a3: wv(2x128) mk0(128)           -> V path + tile-0 mask
#   b:  xt1(2x128) mk1(128) xtW(2x256) mk2(128) mk3(128)
OFF_A1_WK = 0
OFF_A1_XT0 = 2 * 128
OFF_A1_B3 = 4 * 128
CRITA1_COLS = 4 * 128 + 3
OFF_A2_WQ = 0
OFF_A2_XQ = 2 * 128
CRITA2_COLS = 4 * 128
OFF_A3_WV = 0
OFF_A3_MK0 = 2 * 128
CRITA3_COLS = 3 * 128
OFF_B_XT1 = 0
OFF_B_MK1 = 2 * 128
OFF_B_XTW = 3 * 128          # xt cols 256:512 (tile-3 half j-reversed), 2 D-halves
OFF_B_MK2 = 3 * 128 + 512
OFF_B_MK3 = 4 * 128 + 512
CRITB_COLS = 5 * 128 + 512
# groups whose DIAGONAL tile is produced on ACT (per-row scalar.mul)
# with the diag DMA issued on the scalar HWDGE ring: ACT's ~3.4us/tile
# latency then never blocks the sync ring's FIFO of DVE-produced DMAs.
# Full tiles always go to DVE (1.13us/tile batched).
ACT_DIAG = frozenset({4, 5, 6, 7, 8, 9, 12, 13, 14, 15})


def _rows_sel(k: int) -> np.ndarray:
    # 8-row groups: local group g = 4*t + s (t = j-tile class, s = slot)
    # holds global rows 128*t + 32*s + 8*k .. +8.  Slot s's diagonal-tile
    # write allowance np = 32*(s+1) covers quarter k's rows for every k,
    # so one instruction stream serves all cores while the diagonal tile
    # is written trimmed (j < np only; the rest stays pre-zeroed).
    return np.concatenate(
        [
            np.arange(128 * t + 32 * s + 8 * k, 128 * t + 32 * s + 8 * k + 8)
            for t in range(4)
            for s in range(4)
        ]
    )


def _build_nc() -> bass.Bass:
    nc = bass.Bass(trn_type="TRN2")

    crit_a1 = nc.dram_tensor("crit_a1", [128, CRITA1_COLS], F32R, kind="ExternalInput")
    crit_a2 = nc.dram_tensor("crit_a2", [128, CRITA2_COLS], F32R, kind="ExternalInput")
    crit_a3 = nc.dram_tensor("crit_a3", [128, CRITA3_COLS], F32R, kind="ExternalInput")
    crit_b = nc.dram_tensor("crit_b", [128, CRITB_COLS], F32R, kind="ExternalInput")
    # local output layout [j, i_local, p]: per-DMA-partition runs are
    # (i,p)-contiguous (4 KB per 8-row group) instead of 512 B
    out = nc.dram_tensor("out", [S, 128, P], F32, kind="ExternalOutput")

    with TileContext(nc) as tc:
        with (
            tc.tile_pool(name="const", bufs=1) as cpool,
            tc.tile_pool(name="psA", bufs=1, space="PSUM") as psA,
            tc.tile_pool(name="psW", bufs=2, space="PSUM") as psW,
            tc.tile_pool(name="psB", bufs=2, space="PSUM") as psB,
            tc.tile_pool(name="slab", bufs=3) as spool,
            tc.tile_pool(name="dslab", bufs=4) as dpool,
        ):
            # ---- input loads ----
            # THREE packed HWDGE DMAs on the sync ring, staged so each
            # dependency chain unblocks as early as possible.  No SWDGE:
            # gpsimd sem latency (~2.5us after data) once stalled the
            # Tensor queue because the Tile scheduler slotted wide
            # matmuls ahead of the class-0 chain.
            critA1_sb = cpool.tile([128, CRITA1_COLS], F32R, tag="critA1")
            nc.sync.dma_start(critA1_sb[:], crit_a1[:])
            critA2_sb = cpool.tile([128, CRITA2_COLS], F32R, tag="critA2")
            nc.sync.dma_start(critA2_sb[:], crit_a2[:])
            critA3_sb = cpool.tile([128, CRITA3_COLS], F32R, tag="critA3")
            nc.sync.dma_start(critA3_sb[:], crit_a3[:])
            critB_sb = cpool.tile([128, CRITB_COLS], F32R, tag="critB")
            nc.sync.dma_start(critB_sb[:], crit_b[:])

            def a1slice(idx, n=128):
                return critA1_sb[:, idx : idx + n]

            def a2slice(idx, n=128):
                return critA2_sb[:, idx : idx + n]

            def a3slice(idx, n=128):
                return critA3_sb[:, idx : idx + n]

            def bslice(idx, n=128):
                return critB_sb[:, idx : idx + n]

            bq_c = a1slice(OFF_A1_B3, 1).bitcast(F32)
            bk_c = a1slice(OFF_A1_B3 + 1, 1).bitcast(F32)
            bv_c = a1slice(OFF_A1_B3 + 2, 1).bitcast(F32)

            def wqk(which, c):  # 0=q, 1=k
                if which == 0:
                    return a2slice(OFF_A2_WQ + c * P, P)
                return a1slice(OFF_A1_WK + c * P, P)

            def wv(c):
                return a3slice(OFF_A3_WV + c * P, P)

            identity = cpool.tile([128, 128], F32, tag="ident")
            nc.vector.memset(identity[:], 0.0)
            make_identity(nc, identity[:], nomemset=True)

            kt_t = [None] * 4
            v_t = [None] * 4
            stm_t = [None] * 4

            def make_k(jt: int, xt0_ap, xt1_ap):
                ktp = psB.tile([128, 128], F32, tag="proj", name=f"ktp{jt}")
                nc.tensor.matmul(ktp[:], wqk(1, 0), xt0_ap, start=True, stop=False)
                nc.tensor.matmul(ktp[:], wqk(1, 1), xt1_ap, start=False, stop=True)
                ktile = cpool.tile([128, 128], F32R, tag=f"kt{jt}", name=f"kt{jt}")
                nc.vector.tensor_scalar_add(ktile[:], ktp[:], bk_c)
                kt_t[jt] = ktile

            # K^T tile 0 first: it depends only on crit_a1, the first
            # and smallest input DMA, so the Tensor engine starts ~1us
            # before crit_a2 (Q/V path) lands.
            make_k(0, a1slice(OFF_A1_XT0), a1slice(OFF_A1_XT0 + 128))

            # Q^T [p, i].  tile_wait_until pins the SIM-side earliest
            # start of each input-DMA-gated stage to its measured HW sem
            # time (the scheduler's DMA model undershoots completion
            # latency ~2us, which otherwise lets it slot later-gated
            # matmuls ahead of the ramp-critical chain in the static
            # engine queues).
            with tc.tile_wait_until(0.004):
                qt_ps = psA.tile([128, 128], F32, tag="qtps")
                nc.tensor.matmul(qt_ps[:], wqk(0, 0), a2slice(OFF_A2_XQ), start=True, stop=False)
                nc.tensor.matmul(qt_ps[:], wqk(0, 1), a2slice(OFF_A2_XQ + 128), start=False, stop=True)
                qt_sb = cpool.tile([128, 128], F32R, tag="qt")
                nc.vector.tensor_scalar_add(qt_sb[:], qt_ps[:], bq_c)

            def make_narrow(jt: int, xt0_ap, xt1_ap, mk_ap):
                if kt_t[jt] is None:
                    make_k(jt, xt0_ap, xt1_ap)
                ktile = kt_t[jt]

                sp = psB.tile([128, 128], F32, tag="proj", name=f"sps{jt}")
                nc.tensor.matmul(sp[:], qt_sb[:], ktile[:], start=True, stop=True)
                st = cpool.tile([128, 128], F32, tag=f"st{jt}", name=f"st{jt}")
                nc.scalar.activation(
                    st[:], sp[:], mybir.ActivationFunctionType.Sigmoid,
                    scale=INV_SQRT_P,
                )
                stp = psB.tile([128, 128], F32, tag="tp", name=f"stp{jt}")
                nc.tensor.transpose(stp[:], st[:], identity[:])
                stm = cpool.tile([128, 128], F32, tag=f"stm{jt}", name=f"stm{jt}")
                nc.vector.tensor_mul(stm[:], stp[:], mk_ap)
                stm_t[jt] = stm

                vtp = psB.tile([128, 128], F32, tag="proj", name=f"vtp{jt}")
                nc.tensor.matmul(vtp[:], wv(0), xt0_ap, start=True, stop=False)
                nc.tensor.matmul(vtp[:], wv(1), xt1_ap, start=False, stop=True)
                vT = cpool.tile([128, 128], F32, tag=f"vT{jt}", name=f"vT{jt}")
                nc.scalar.add(vT[:], vtp[:], bv_c)
                vp = psB.tile([128, 128], F32, tag="tp", name=f"vp{jt}")
                nc.tensor.transpose(vp[:], vT[:], identity[:])
                vt = cpool.tile([128, P], F32, tag=f"v{jt}", name=f"v{jt}")
                nc.scalar.copy(vt[:], vp[:])
                v_t[jt] = vt

            def make_wide():
                # tiles 2-3 in one N=256 fp32r pass each
                ktpR = psW.tile([128, 256], F32, tag="wide", name="ktpR")
                nc.tensor.matmul(ktpR[:], wqk(1, 0), bslice(OFF_B_XTW, 256), start=True, stop=False)
                nc.tensor.matmul(ktpR[:], wqk(1, 1), bslice(OFF_B_XTW + 256, 256), start=False, stop=True)
                ktR = cpool.tile([128, 256], F32R, tag="ktR")
                nc.scalar.add(ktR[:], ktpR[:], bk_c)

                spR = psW.tile([128, 256], F32, tag="wide", name="spR")
                nc.tensor.matmul(spR[:], qt_sb[:], ktR[:], start=True, stop=True)
                stR = cpool.tile([128, 256], F32, tag="stR")
                nc.scalar.activation(
                    stR[:], spR[:], mybir.ActivationFunctionType.Sigmoid,
                    scale=INV_SQRT_P,
                )
                vtpR = psW.tile([128, 256], F32, tag="wide", name="vtpR")
                nc.tensor.matmul(vtpR[:], wv(0), bslice(OFF_B_XTW, 256), start=True, stop=False)
                nc.tensor.matmul(vtpR[:], wv(1), bslice(OFF_B_XTW + 256, 256), start=False, stop=True)
                vTR = cpool.tile([128, 256], F32, tag="vTR")
                nc.scalar.add(vTR[:], vtpR[:], bv_c)

                for jt in (2, 3):
                    c = jt - 2
                    stp = psB.tile([128, 128], F32, tag="tp", name=f"stp{jt}")
                    nc.tensor.transpose(stp[:], stR[:, c * 128 : (c + 1) * 128], identity[:])
                    stm = cpool.tile([128, 128], F32, tag=f"stm{jt}", name=f"stm{jt}")
                    nc.vector.tensor_mul(
                        stm[:], stp[:], bslice(OFF_B_MK2 + c * 128).bitcast(F32)
                    )
                    stm_t[jt] = stm
                    vp = psB.tile([128, 128], F32, tag="tp", name=f"vp{jt}")
                    nc.tensor.transpose(vp[:], vTR[:, c * 128 : (c + 1) * 128], identity[:])
                    vt = cpool.tile([128, P], F32, tag=f"v{jt}", name=f"v{jt}")
                    nc.scalar.copy(vt[:], vp[:])
                    v_t[jt] = vt

            # ---- output slab stage ----
            out_r = out.rearrange("(t j) i p -> j t (i p)", j=128)  # [128,4,16384]

            def emit_group(g: int):
                L = g // (NGROUPS // 4) + 1
                np_ = 32 * (g % 4) + 32   # diag-tile partition allowance
                act_diag = g in ACT_DIAG
                slab = spool.tile(
                    [128, L * GROUP * 128], F32, tag=f"slab{L}", name=f"slab_g{g}"
                )
                # ACT's diag goes to its OWN small slab with its own
                # rotation: the big slab is then released by the fast
                # sync-ring full DMAs alone, so DVE never stalls waiting
                # for ACT's ~3.4us/tile production to drain.
                dslab = (
                    dpool.tile([128, GROUP * 128], F32, tag="dslab", name=f"dslab_g{g}")
                    if act_diag
                    else None
                )
                for jt in range(L):
                    dst3 = slab[
                        :, jt * GROUP * 128 : (jt + 1) * GROUP * 128
                    ].rearrange("q (i p) -> q i p", i=GROUP)
                    if act_diag and jt == L - 1:
                        dst3 = dslab[:].rearrange("q (i p) -> q i p", i=GROUP)
                        for ii in range(GROUP):
                            li = g * GROUP + ii
                            nc.scalar.mul(
                                dst3[:, ii, :],
                                v_t[jt][:],
                                mul=stm_t[jt][:, li : li + 1],
                            )
                    else:
                        v3 = v_t[jt][:].unsqueeze(1).broadcast_to([128, GROUP, 128])
                        s3 = (
                            stm_t[jt][:, g * GROUP : (g + 1) * GROUP]
                            .unsqueeze(2)
                            .broadcast_to([128, GROUP, 128])
                        )
                        nc.vector.tensor_mul(dst3, v3, s3)
                grange = slice(GROUP * 128 * g, GROUP * 128 * (g + 1))
                # one DMA per full tile, issued as soon as that tile's
                # slab section is produced - a multi-tile DMA would wait
                # for the whole slab, bubbling the sync ring at class
                # transitions
                for jt in range(L - 1):
                    nc.sync.dma_start(
                        out_r[:, jt : jt + 1, grange],
                        slab[
                            :, jt * GROUP * 128 : (jt + 1) * GROUP * 128
                        ].rearrange("q (t ip) -> q t ip", t=1),
                    )
                # diagonal tile: only np_ partitions can be nonzero; the
                # rest of the shard stays pre-zeroed in HBM.  Odd classes
                # are stored j-reversed (host reverses their xt columns
                # and masks, and un-reverses on gather), putting the
                # trimmed slice at the TOP partitions - this flattens
                # per-partition DMA bytes so no SBUF port runs ~2x hot
                # (uneven ports stall DVE's lockstep lanes ~20%).
                ps = slice(128 - np_, 128) if (L - 1) % 2 else slice(0, np_)
                if act_diag:
                    nc.scalar.dma_start(
                        out_r[ps, L - 1 : L, grange],
                        dslab[ps, :].rearrange("q (t ip) -> q t ip", t=1),
                    )
                else:
                    nc.sync.dma_start(
                        out_r[ps, L - 1 : L, grange],
                        slab[ps, (L - 1) * GROUP * 128 : L * GROUP * 128].rearrange(
                            "q (t ip) -> q t ip", t=1
                        ),
                    )

            with tc.tile_wait_until(0.005):
                make_narrow(
                    0, a1slice(OFF_A1_XT0), a1slice(OFF_A1_XT0 + 128),
                    a3slice(OFF_A3_MK0).bitcast(F32),
                )
            for g in range(0, 4):      # class 0 (tile 0 only)
                emit_group(g)
            with tc.tile_wait_until(0.0077):
                make_narrow(
                    1, bslice(OFF_B_XT1), bslice(OFF_B_XT1 + 128),
                    bslice(OFF_B_MK1).bitcast(F32),
                )
                make_wide()            # early: its chain must be done
                                       # before class-3 groups need it
            for g in range(4, 8):      # class 1 (tiles 0-1)
                emit_group(g)
            for g in range(12, 16):    # class 3
                emit_group(g)
            for g in range(8, 12):     # class 2
                emit_group(g)

    _split_multi_waits(nc)
    return nc


def _split_multi_waits(nc):
    """This toolchain's walrus accepts at most one sync wait per
    instruction; split extras into single-wait NoOps just before the
    instruction on the same engine queue (waits are ANDed preconditions,
    executed in order on the engine's queue — semantically identical)."""
    for fn in nc.m.functions:
        for blk in fn.blocks:
            insts = blk.instructions
            i = 0
            while i < len(insts):
                inst = insts[i]
                si = getattr(inst, "sync_info", None)
                if si is not None and si.on_wait is not None and len(si.on_wait) > 1:
                    waits = list(si.on_wait)
                    nops = [
                        mybir.InstNoOp(
                            name=nc.get_next_instruction_name(),
                            engine=inst.engine,
                            sync_info=mybir.SyncInfo(on_wait=[w], on_update=[]),
                            bass_nofuse=True,
                        )
                        for w in waits[:-1]
                    ]
                    si.on_wait = [waits[-1]]
                    insts[i:i] = nops
                    i += len(nops)
                i += 1


_NC_CACHE = None


def _get_nc():
    global _NC_CACHE
    if _NC_CACHE is None:
        _NC_CACHE = _build_nc()
    return _NC_CACHE


def _in_maps(x_set, Wq, bq, Wk, bk, Wv, bv):
    xts = [
        np.ascontiguousarray(x_set[b].T).astype(np.float32, copy=False)
        for b in range(B)
    ]
    wqT, wkT, wvT = Wq.T, Wk.T, Wv.T
    b3c = np.stack([bq, bk, bv], axis=1).astype(np.float32)
    jj = np.arange(128)
    maps = []
    for c in range(NCORES):
        b, k = divmod(c, 4)
        rows = _rows_sel(k)
        xtT = xts[b]
        xqT = xtT[:, rows]
        mask = np.empty((4, 128, 128), np.float32)
        for jt in range(4):
            mask[jt] = ((jt * 128 + jj)[:, None] <= rows[None, :]).astype(np.float32)
        # odd tiles are stored j-reversed on device (see emit_group)
        mask[1] = mask[1][::-1]
        mask[3] = mask[3][::-1]
        # xt cols 256:512 with the tile-3 half j-reversed, for the wide path
        xw = np.concatenate([xtT[:, 256:384], xtT[:, 511:383:-1]], axis=1)
        crit_a1 = np.concatenate(
            [
                wkT[0:128], wkT[128:256],
                xtT[0:128, 0:128], xtT[128:256, 0:128],
                b3c,
            ],
            axis=1,
        ).astype(np.float32, copy=False)
        crit_a2 = np.concatenate(
            [
                wqT[0:128], wqT[128:256],
                xqT[0:128], xqT[128:256],
            ],
            axis=1,
        ).astype(np.float32, copy=False)
        crit_a3 = np.concatenate(
            [
                wvT[0:128], wvT[128:256],
                mask[0],
            ],
            axis=1,
        ).astype(np.float32, copy=False)
        crit_b = np.concatenate(
            [
                xtT[0:128, 255:127:-1], xtT[128:256, 255:127:-1],
                mask[1],
                xw[0:128], xw[128:256],
                mask[2], mask[3],
            ],
            axis=1,
        ).astype(np.float32, copy=False)
        maps.append(
            {
                "crit_a1": np.ascontiguousarray(crit_a1),
                "crit_a2": np.ascontiguousarray(crit_a2),
                "crit_a3": np.ascontiguousarray(crit_a3),
                "crit_b": np.ascontiguousarray(crit_b),
            }
        )
    return maps


def run(x_set, Wq, bq, Wk, bk, Wv, bv, **spmd_kwargs):
    nc = _get_nc()
    in_maps = _in_maps(x_set, Wq, bq, Wk, bk, Wv, bv)
    res = bass_utils.run_bass_kernel_spmd(
        nc, in_maps, core_ids=list(range(NCORES)), **spmd_kwargs
    )
    full = np.zeros((B, S, S, P), np.float32)
    for c in range(NCORES):
        b, k = divmod(c, 4)
        # core output is [(t j), i_local, p]; odd tiles are stored
        # j-reversed on device -> flip them back, then scatter as
        # [i_local, j, p]
        o = res.results[c]["out"].reshape(4, 128, 128, P)
        o = np.concatenate([o[0:1], o[1:2, ::-1], o[2:3], o[3:4, ::-1]])
        full[b, _rows_sel(k)] = o.reshape(S, 128, P).transpose(1, 0, 2)
    return full, res


def kernel(x_set, Wq, bq, Wk, bk, Wv, bv):
    full, _ = run(x_set, Wq, bq, Wk, bk, Wv, bv)
    return full

